# revision 21
# baseline (speedup 1.0000x reference)
"""GroupViT cross-attention layer on 8 TRN2 NeuronCores.

Data-parallel over batch (16 batches -> 2 per core, zero collectives).
Feature-major layout on chip: activations [feature(partition), token(free)],
weights host-transposed so every matmul contracts over the partition dim.

v2: fp8(e4m3) DoubleRow matmuls for the Q/K/V/out projections and the
ctx (probs @ V) matmul -- DoubleRow contracts 256 rows per instruction,
halving PE instruction count vs bf16.  Scores stay bf16 (K=64 per head
cannot exploit DoubleRow).  MLP bf16.  Host converts query/key/weights
to fp8/bf16 so no on-chip casts of the big operands are needed.

Phase-serial schedule: [attn b0, attn b1] under the exp ACT table, then
[LN2 b0,b1][MLP b0,b1][LNp b0,b1] under rsqrt/gelu tables -- 4 ACT
table loads total instead of ~30.  Softmax reciprocal runs on DVE
(reciprocal_approx_fast); its broadcast across 64 rows is a k=1 matmul
into the unused upper partitions of the same ctx PSUM bank.
"""

import numpy as np
import ml_dtypes

B, T, S, D, H, HD, FF = 16, 512, 2048, 768, 12, 64, 3072
NCORES = 8
BPC = B // NCORES
P = 128
DC = D // P            # 6 feature chunks
KP = DC // 2           # 3 doublerow k-pairs
SC = S // P            # 16 key-token chunks
SP = SC // 2           # 8 key-token chunk pairs
FFC = FF // P          # 24
EPS = 1e-5
SCALE = HD ** -0.5

MLP_FP8 = False        # fc1/fc2 in fp8 DoubleRow

_cached = {}


def _build(use_bv: bool, mlp_fp8: bool):
    import concourse.bacc as bacc
    import concourse.tile as tile
    import concourse.mybir as mybir

    f32 = mybir.dt.float32
    f32r = mybir.dt.float32r
    bf16 = mybir.dt.bfloat16
    fp8 = mybir.dt.float8e4
    AF = mybir.ActivationFunctionType
    ALU = mybir.AluOpType
    DR = mybir.MatmulPerfMode.DoubleRow

    nc = bacc.Bacc("TRN2", target_bir_lowering=False, debug=False,
                   num_devices=NCORES)

    # ---- DRAM I/O (per-core shapes, host pre-tiled) ----
    qT8_d = nc.dram_tensor("qT8", [BPC, P, KP, 2, T], fp8, kind="ExternalInput")
    qTf_d = nc.dram_tensor("qTf", [BPC, P, DC, T], f32r, kind="ExternalInput")
    kT8_d = nc.dram_tensor("kT8", [BPC, P, KP, 2, S], fp8, kind="ExternalInput")
    w8q_d = nc.dram_tensor("w8q", [P, KP, 2, D], fp8, kind="ExternalInput")
    w8k_d = nc.dram_tensor("w8k", [P, KP, 2, D], fp8, kind="ExternalInput")
    w8v_d = nc.dram_tensor("w8v", [P, KP, 2, D], fp8, kind="ExternalInput")
    w8o_d = nc.dram_tensor("w8o", [P, KP, 2, D], fp8, kind="ExternalInput")
    if mlp_fp8:
        fc1_d = nc.dram_tensor("fc1_t", [FFC, P, KP, 2, P], fp8,
                               kind="ExternalInput")
        fc2_d = nc.dram_tensor("fc2_t", [FFC // 2, P, 2, D], fp8,
                               kind="ExternalInput")
    else:
        fc1_d = nc.dram_tensor("fc1_t", [FFC, P, DC, P], bf16,
                               kind="ExternalInput")
        fc2_d = nc.dram_tensor("fc2_t", [FFC, P, D], bf16,
                               kind="ExternalInput")
    bq_d = nc.dram_tensor("bqv", [P, DC], f32, kind="ExternalInput")
    bk_d = nc.dram_tensor("bkv", [P, DC], f32, kind="ExternalInput")
    bo_d = nc.dram_tensor("bov", [P, DC], f32, kind="ExternalInput")
    bv_d = nc.dram_tensor("bvv", [1, D], f32r, kind="ExternalInput")
    f1b_d = nc.dram_tensor("f1b", [P, FFC], f32, kind="ExternalInput")
    f2b_d = nc.dram_tensor("f2b", [P, DC], f32, kind="ExternalInput")
    # LN rows packed [4, D] bf16: 0=ln2g 1=-ln2b 2=lnpg 3=-lnpb
    lnrows_d = nc.dram_tensor("lnrows", [4, D], bf16, kind="ExternalInput")
    ones_col_d = nc.dram_tensor("ones_col", [P, 1], f32r, kind="ExternalInput")
    ones_row_d = nc.dram_tensor("ones_row", [1, T], f32r, kind="ExternalInput")
    out_d = nc.dram_tensor("out", [BPC, P, DC, T], f32, kind="ExternalOutput")

    def F(ap):
        return ap.bitcast(f32)

    from contextlib import ExitStack

    with tile.TileContext(nc) as tc:
        with ExitStack() as stack:
            pool_specs = [
                ("small", 1), ("wres", 1), ("kinp", 2), ("qinp", 2),
                ("qfp", 2), ("qtp", 2), ("ktc", 2), ("vp", 2), ("expp", 3),
                ("ctxp", 2), ("xp", 2), ("hp", 2), ("outp", 2),
                ("sqp", 2), ("mchp", 2), ("fstream", 2), ("rdp", 1),
                ("lnr", 1), ("tmp", 2),
            ]
            pools = {nm: stack.enter_context(tc.tile_pool(name=nm, bufs=bu))
                     for nm, bu in pool_specs}
            (small, wres, kinp, qinp, qfp, qtp, ktc, vp, expp, ctxp, xp,
             hp, outp, sqp, mchp, fstream, rdp, lnr, tmpp) = (
                pools[nm] for nm, _ in pool_specs)
            # ---- persistent small tiles ----
            ones_col = small.tile([P, 1], f32r, tag="ones_col")
            nc.sync.dma_start(ones_col[:], ones_col_d.ap())
            ones_row = small.tile([1, T], f32r, tag="ones_row")
            nc.sync.dma_start(ones_row[:], ones_row_d.ap())
            ones_row_bf = small.tile([1, T], bf16, tag="ones_row_bf")
            nc.vector.tensor_copy(ones_row_bf[:], F(ones_row[:]))
            m1 = small.tile([2, T], bf16, tag="lnm1")
            nc.sync.dma_start(m1[1:2, :], ones_row_bf[:])
            o64f = small.tile([1, HD], f32, tag="ones64f")
            nc.vector.memset(o64f[:], 1.0)
            ones64_bf = small.tile([1, HD], bf16, tag="ones64")
            nc.vector.tensor_copy(ones64_bf[:], o64f[:])
            onesc_f = small.tile([P, 1], f32, tag="onesc_f")
            nc.vector.memset(onesc_f[:], 1.0)
            eps_t = small.tile([1, 1], f32, tag="eps")
            nc.vector.memset(eps_t[:], EPS)

            ln2gb = small.tile([2, D], bf16, tag="ln2gb")
            nc.sync.dma_start(ln2gb[:], lnrows_d.ap()[0:2, :])
            lnpgb = small.tile([2, D], bf16, tag="lnpgb")
            nc.sync.dma_start(lnpgb[:], lnrows_d.ap()[2:4, :])

            bq_pc = small.tile([P, DC], f32, tag="bq_pc")
            nc.sync.dma_start(bq_pc[:], bq_d.ap())
            bk_pc = small.tile([P, DC], f32, tag="bk_pc")
            nc.sync.dma_start(bk_pc[:], bk_d.ap())
            bo_pc = small.tile([P, DC], f32, tag="bo_pc")
            nc.sync.dma_start(bo_pc[:], bo_d.ap())
            f1b_pc = small.tile([P, FFC], f32, tag="f1b_pc")
            nc.sync.dma_start(f1b_pc[:], f1b_d.ap())
            f2b_pc = small.tile([P, DC], f32, tag="f2b_pc")
            nc.sync.dma_start(f2b_pc[:], f2b_d.ap())

            bv_row = None
            if use_bv:
                bv_row = small.tile([1, D], f32r, tag="bv_row")
                nc.sync.dma_start(bv_row[:], bv_d.ap())

            # ---- resident fp8 attention weights ----
            w8 = {}
            for nm, dram in (("q", w8q_d), ("k", w8k_d), ("v", w8v_d),
                             ("o", w8o_d)):
                t8 = wres.tile([P, KP, 2, D], fp8, tag=f"w8{nm}")
                nc.gpsimd.dma_start(t8[:], dram.ap())
                w8[nm] = t8

            # ---- per-batch persistent activations ----
            kin, qin8, qinf, qt, v8, ctx8, xT, hT, x2T = \
                {}, {}, {}, {}, {}, {}, {}, {}, {}

            def attn(b, psBIG, psSC, psCTX, psBC):
                kin[b] = kinp.tile([P, KP, 2, S], fp8, tag="kin", name=f"kin{b}")
                nc.gpsimd.dma_start(kin[b][:], kT8_d.ap()[b])
                qin8[b] = qinp.tile([P, KP, 2, T], fp8, tag="qin8", name=f"qin8_{b}")
                nc.sync.dma_start(qin8[b][:], qT8_d.ap()[b])

                # ---- Q projection (fp8 DR) -> qt bf16 [128, 6, 2, T]
                # head-h slice lives in its 64 rows, other 64 rows zero,
                # so scores contract K=128 against the full ktch chunk.
                qt[b] = qtp.tile([P, DC, 2, T], bf16, tag="qt", name=f"qt{b}")
                nc.vector.memset(qt[b][0:HD, :, 1, :], 0.0)
                nc.vector.memset(qt[b][HD:P, :, 0, :], 0.0)
                for mo in range(DC):
                    ps = psBIG.tile([P, T], f32, tag="psBIG")
                    for kp in range(KP):
                        nc.tensor.matmul(
                            ps[:], w8["q"][:, kp, :, mo * P:(mo + 1) * P],
                            qin8[b][:, kp, :, :],
                            start=(kp == 0), stop=(kp == KP - 1), perf_mode=DR)
                    nc.vector.tensor_scalar_add(
                        qt[b][0:HD, mo, 0, :], ps[0:HD, :],
                        bq_pc[0:HD, mo:mo + 1])
                    nc.vector.tensor_scalar_add(
                        qt[b][HD:P, mo, 1, :], ps[HD:P, :],
                        bq_pc[HD:P, mo:mo + 1])

                ktchs = {}

                def kproj_pre(c):
                    ktch = ktc.tile([P, S], bf16, tag="ktc", name=f"ktc{c}")
                    for st in range(4):
                        ps = psBIG.tile([P, T], f32, tag="psBIG")
                        for kp in range(KP):
                            nc.tensor.matmul(
                                ps[:], w8["k"][:, kp, :, c * P:(c + 1) * P],
                                kin[b][:, kp, :, st * T:(st + 1) * T],
                                start=(kp == 0), stop=(kp == KP - 1),
                                perf_mode=DR)
                        nc.vector.tensor_scalar_add(
                            ktch[:, st * T:(st + 1) * T], ps[:],
                            bk_pc[:, c:c + 1])
                    return ktch

                # ---- V projection (fp8 DR) -> v8 [128, 8, 2, 12, 65] ----
                v8[b] = vp.tile([P, SP, 2, H, HD + 4], fp8, tag="v8", name=f"v8_{b}")
                vflat = v8[b][:].rearrange("p a b h e -> p (a b h) e")
                nc.vector.memset(vflat[:, :, HD + 1:HD + 4], 0.0)
                nc.vector.tensor_copy(
                    vflat[:, :, HD:HD + 1],
                    onesc_f[:].to_broadcast([P, SP * 2 * H, 1]))
                bv_bc = None
                if use_bv:
                    bv_bc = small.tile([P, D], f32, tag="bv_bc")
                    for half in range(2):
                        ps = psBIG.tile([P, T], f32, tag="psBIG")
                        nc.tensor.matmul(
                            ps[:, 0:384], ones_row[:, 0:P],
                            bv_row[:, half * 384:(half + 1) * 384],
                            start=True, stop=True)
                        nc.vector.tensor_copy(
                            bv_bc[:, half * 384:(half + 1) * 384], ps[:, 0:384])
                for so in range(SC):
                    if so == 0:
                        ktchs[0] = kproj_pre(0)
                    if so == 8:
                        ktchs[1] = kproj_pre(1)
                    for half in range(2):
                        ps = psBIG.tile([P, T], f32, tag="psBIG")
                        for kp in range(KP):
                            nc.tensor.matmul(
                                ps[:, 0:384],
                                kin[b][:, kp, :, so * P:(so + 1) * P],
                                w8["v"][:, kp, :, half * 384:(half + 1) * 384],
                                start=(kp == 0), stop=(kp == KP - 1),
                                perf_mode=DR)
                        dstv = v8[b][:, so // 2, so % 2,
                                     6 * half:6 * half + 6, 0:HD]
                        if use_bv:
                            nc.vector.tensor_tensor(
                                dstv, ps[:, 0:384],
                                bv_bc[:, half * 384:(half + 1) * 384], ALU.add)
                        else:
                            nc.vector.tensor_copy(dstv, ps[:, 0:384])

                # ---- attention per feature-chunk (2 heads) ----
                ctx8[b] = ctxp.tile([P, KP, 2, T], fp8, tag="ctx8", name=f"ctx8_{b}")
                for c in range(DC):
                    ktch = ktchs.pop(c) if c in ktchs else kproj_pre(c)
                    if c + 1 < DC and c >= 1:
                        ktchs[c + 1] = kproj_pre(c + 1)

                    ps_ctx = [psCTX.tile([P, T], f32, tag="psCTX",
                                         name=f"ps_ctx{i}") for i in range(2)]
                    for sp in range(SP):
                        pscs = [psSC.tile([P, 2, T], f32, tag="psSC",
                                          name=f"ps_sc{i}") for i in range(2)]
                        for j in range(2):
                            so = sp * 2 + j
                            for hh in range(2):
                                nc.tensor.matmul(
                                    pscs[hh][:, j, :],
                                    ktch[:, so * P:(so + 1) * P],
                                    qt[b][:, c, hh, :],
                                    start=True, stop=True)
                        exs = []
                        for hh in range(2):
                            ex = expp.tile([P, 2, T], fp8, tag="exp",
                                           name=f"ex{hh}")
                            nc.scalar.activation(ex[:], pscs[hh][:], AF.Exp)
                            exs.append(ex)
                        for hh in range(2):
                            h = 2 * c + hh
                            for j in range(2):
                                nc.tensor.matmul(
                                    ps_ctx[hh][0:HD + 4, :],
                                    v8[b][:, sp, j, h, :],
                                    exs[hh][:, j, :],
                                    start=(sp == 0 and j == 0),
                                    stop=(sp == SP - 1 and j == 1))
                    for hh in range(2):
                        h = 2 * c + hh
                        den_sb = rdp.tile([1, T], f32, tag="den_sb")
                        nc.vector.tensor_copy(den_sb[:],
                                              ps_ctx[hh][HD:HD + 1, :])
                        rden = rdp.tile([1, T], f32, tag="rden")
                        nc.vector.reciprocal_approx_fast(
                            out=rden[:], in_=den_sb[:])
                        rden_bf = rdp.tile([1, T], bf16, tag="rden_bf")
                        nc.vector.tensor_copy(rden_bf[:], rden[:])
                        ps_bc = psBC.tile([HD, T], f32, tag="psBC")
                        nc.tensor.matmul(ps_bc[:], ones64_bf[:],
                                         rden_bf[:], start=True, stop=True)
                        bc_sb = tmpp.tile([HD, T], bf16, tag="bc_sb")
                        nc.vector.tensor_copy(bc_sb[:], ps_bc[:])
                        nc.vector.tensor_tensor(
                            ctx8[b][(h % 2) * HD:(h % 2) * HD + HD,
                                    (h // 2) // 2, (h // 2) % 2, :],
                            ps_ctx[hh][0:HD, :], bc_sb[:], ALU.mult)

                # ---- out projection (fp8 DR) + residual -> xT f32r ----
                xT[b] = xp.tile([P, DC, T], f32r, tag="xT", name=f"xT{b}")
                for mo in range(DC):
                    qf = qfp.tile([P, T], f32r, tag="qinf")
                    nc.sync.dma_start(qf[:], qTf_d.ap()[b][:, mo, :])
                    ps = psBIG.tile([P, T], f32, tag="psBIG")
                    for kp in range(KP):
                        nc.tensor.matmul(
                            ps[:], w8["o"][:, kp, :, mo * P:(mo + 1) * P],
                            ctx8[b][:, kp, :, :],
                            start=(kp == 0), stop=(kp == KP - 1), perf_mode=DR)
                    nc.vector.scalar_tensor_tensor(
                        xT[b][:, mo, :], ps[:], bo_pc[:, mo:mo + 1],
                        F(qf[:]), op0=ALU.add, op1=ALU.add)

            def ln_pass(xsrc, gb_pair, ps_st, ps_bc, dst_alloc):
                """LayerNorm over the partition(feature) dim.
                xsrc [P, DC, T] f32r; dst_alloc(c2) -> (dst_ap, finish|None)."""
                psum_mu = ps_st.tile([1, T], f32, tag="st_mu")
                psum_sq = ps_st.tile([1, T], f32, tag="st_sq")
                for c2 in range(DC):
                    nc.tensor.matmul(psum_mu[:], ones_col[:], xsrc[:, c2, :],
                                     start=(c2 == 0), stop=(c2 == DC - 1))
                sqt = []
                for c2 in range(DC):
                    sq = sqp.tile([P, T], f32r, tag="lnsq")
                    nc.vector.tensor_mul(sq[:], F(xsrc[:, c2, :]),
                                         F(xsrc[:, c2, :]))
                    sqt.append(sq)
                for c2 in range(DC):
                    nc.tensor.matmul(psum_sq[:], ones_col[:], sqt[c2][:],
                                     start=(c2 == 0), stop=(c2 == DC - 1))
                mu_t = lnr.tile([1, T], f32, tag="lnmu")
                mu2_t = lnr.tile([1, T], f32, tag="lnmu2")
                rs_t = lnr.tile([1, T], bf16, tag="lnrs")
                mrs_t = lnr.tile([1, T], bf16, tag="lnmrs")
                mu_f, mu2_f = mu_t[:], mu2_t[:]
                rs_f, mrs_f = rs_t[:], mrs_t[:]
                nc.vector.tensor_scalar_mul(mu_f, psum_mu[:], 1.0 / D)
                nc.vector.tensor_tensor(mu2_f, mu_f, mu_f, ALU.mult)
                var_f = mu2_f
                nc.vector.scalar_tensor_tensor(
                    var_f, psum_sq[:], 1.0 / D, mu2_f,
                    op0=ALU.mult, op1=ALU.subtract)
                nc.scalar.activation(rs_f, var_f, AF.Abs_reciprocal_sqrt,
                                     bias=eps_t[:])
                nc.vector.tensor_tensor(mrs_f, mu_f, rs_f, ALU.mult)
                nc.vector.tensor_copy(m1[0:1, :], mrs_f)
                for c2 in range(DC):
                    bcA = ps_bc.tile([P, T], f32, tag="ln_bcA")
                    bcB = ps_bc.tile([P, T], f32, tag="ln_bcB")
                    gsl = gb_pair[0:1, c2 * P:(c2 + 1) * P]
                    gbsl = gb_pair[:, c2 * P:(c2 + 1) * P]
                    nc.tensor.matmul(bcA[:], gsl, rs_f,
                                     start=True, stop=True)
                    nc.tensor.matmul(bcB[:], gbsl, m1[:],
                                     start=True, stop=True)
                    dst, finish = dst_alloc(c2)
                    tmp = tmpp.tile([P, T], f32, tag="ln_tmp")
                    nc.vector.tensor_tensor(tmp[:], F(xsrc[:, c2, :]), bcA[:],
                                            ALU.mult)
                    nc.vector.tensor_tensor(dst, tmp[:], bcB[:], ALU.subtract)
                    if finish is not None:
                        finish()

            def mlp(b, psF1, psF2):
                x2T[b] = xT[b]
                ps_f2 = [psF2.tile([P, T], f32, tag="psF2", name=f"ps_f2_{i}")
                         for i in range(DC)]
                if mlp_fp8:
                    hview = hT[b][:]
                    mch = None
                    for fo in range(FFC):
                        f1_sl = fstream.tile([P, KP, 2, P], fp8, tag="f1_sl")
                        nc.sync.dma_start(f1_sl[:], fc1_d.ap()[fo])
                        if fo % 2 == 0:
                            f2_sl = fstream.tile([P, 2, D], fp8, tag="f2_sl")
                            nc.sync.dma_start(f2_sl[:], fc2_d.ap()[fo // 2])
                            mch = mchp.tile([P, 2, T], fp8, tag="mch")
                        ps1 = psF1.tile([P, T], f32, tag="psF1")
                        for kp in range(KP):
                            nc.tensor.matmul(
                                ps1[:], f1_sl[:, kp, :, :], hview[:, kp, :, :],
                                start=(kp == 0), stop=(kp == KP - 1),
                                perf_mode=DR)
                        nc.scalar.activation(mch[:, fo % 2, :], ps1[:],
                                             AF.Gelu, bias=f1b_pc[:, fo:fo + 1])
                        if fo % 2 == 1:
                            for mo in range(DC):
                                nc.tensor.matmul(
                                    ps_f2[mo][:],
                                    f2_sl[:, :, mo * P:(mo + 1) * P], mch[:],
                                    start=(fo == 1), stop=(fo == FFC - 1),
                                    perf_mode=DR)
                else:
                    for fo in range(FFC):
                        f1_sl = fstream.tile([P, DC, P], bf16, tag="f1_sl")
                        nc.sync.dma_start(f1_sl[:], fc1_d.ap()[fo])
                        f2_sl = fstream.tile([P, D], bf16, tag="f2_sl")
                        nc.sync.dma_start(f2_sl[:], fc2_d.ap()[fo])
                        ps1 = psF1.tile([P, T], f32, tag="psF1")
                        for ki in range(DC):
                            nc.tensor.matmul(ps1[:], f1_sl[:, ki, :],
                                             hT[b][:, ki, :],
                                             start=(ki == 0),
                                             stop=(ki == DC - 1))
                        mch = mchp.tile([P, T], bf16, tag="mch")
                        nc.scalar.activation(mch[:], ps1[:], AF.Gelu,
                                             bias=f1b_pc[:, fo:fo + 1])
                        for mo in range(DC):
                            nc.tensor.matmul(
                                ps_f2[mo][:], f2_sl[:, mo * P:(mo + 1) * P],
                                mch[:],
                                start=(fo == 0), stop=(fo == FFC - 1))
                for mo in range(DC):
                    nc.vector.scalar_tensor_tensor(
                        x2T[b][:, mo, :], ps_f2[mo][:], f2b_pc[:, mo:mo + 1],
                        F(xT[b][:, mo, :]), op0=ALU.add, op1=ALU.add)

            # ================= schedule =================
            with (
                tc.tile_pool(name="psBIG", bufs=1, space="PSUM") as psBIG,
                tc.tile_pool(name="psSC", bufs=2, space="PSUM") as psSC,
                tc.tile_pool(name="psCTX", bufs=2, space="PSUM") as psCTX,
                tc.tile_pool(name="psBC", bufs=1, space="PSUM") as psBC,
            ):
                for b in range(BPC):
                    attn(b, psBIG, psSC, psCTX, psBC)

            ln2g, ln2bn = ln2gb[:], ln2gb[:]
            lnpg, lnpbn = lnpgb[:], lnpgb[:]

            with (
                tc.tile_pool(name="psST", bufs=1, space="PSUM") as psST,
                tc.tile_pool(name="psLB", bufs=2, space="PSUM") as psLB,
            ):
                for b in range(BPC):
                    if mlp_fp8:
                        hT[b] = hp.tile([P, KP, 2, T], fp8, tag="hT", name=f"hT{b}")
                        hview = hT[b][:].rearrange("p a b t -> p (a b) t")
                    else:
                        hT[b] = hp.tile([P, DC, T], bf16, tag="hT", name=f"hT{b}")
                        hview = hT[b][:]
                    ln_pass(xT[b], ln2g, psST, psLB,
                            lambda c2, hv=hview: (hv[:, c2, :], None))

            with (
                tc.tile_pool(name="psF1", bufs=2, space="PSUM") as psF1,
                tc.tile_pool(name="psF2", bufs=6, space="PSUM") as psF2,
            ):
                for b in range(BPC):
                    mlp(b, psF1, psF2)

            with (
                tc.tile_pool(name="psST2", bufs=1, space="PSUM") as psST2,
                tc.tile_pool(name="psLB2", bufs=2, space="PSUM") as psLB2,
            ):
                for b in range(BPC):
                    def out_alloc(c2, b=b):
                        t = outp.tile([P, T], f32, tag="outT")
                        fin = (lambda t=t, c2=c2, b=b:
                               nc.sync.dma_start(out_d.ap()[b][:, c2, :], t[:]))
                        return t[:], fin
                    ln_pass(x2T[b], lnpg, psST2, psLB2, out_alloc)

    nc.compile()
    return nc


def _get_nc(use_bv: bool, mlp_fp8: bool):
    key = ("nc", use_bv, mlp_fp8)
    if key not in _cached:
        _cached[key] = _build(use_bv, mlp_fp8)
    return _cached[key]


def _to_fp8(x):
    return np.asarray(x, np.float32).astype(ml_dtypes.float8_e4m3)


def _to_bf16(x):
    return np.asarray(x, np.float32).astype(ml_dtypes.bfloat16)


def _tile_kp(wT):
    """[d_in, n] -> [P, KP, 2, n] with d_in = (kp*2 + i)*P + p."""
    n = wT.shape[1]
    return np.ascontiguousarray(wT.reshape(KP, 2, P, n).transpose(2, 0, 1, 3))


def _col_pc(v, nch):
    """[n] -> [P, nch] with n = c*P + p."""
    return np.ascontiguousarray(np.asarray(v, np.float32).reshape(nch, P).T)


def _prep_shared(wq, bq, wk, bk, wv, bv, wo, bo,
                 ln2_g, ln2_b, fc1_w, fc1_b, fc2_w, fc2_b, lnp_g, lnp_b,
                 mlp_fp8):
    f = np.float32
    c = np.ascontiguousarray
    sc = np.float32(SCALE)
    lnrows = np.stack([
        np.asarray(ln2_g, f), -np.asarray(ln2_b, f),
        np.asarray(lnp_g, f), -np.asarray(lnp_b, f)])
    shared = {
        "w8q": _to_fp8(_tile_kp(np.asarray(wq, f).T * sc)),
        "w8k": _to_fp8(_tile_kp(np.asarray(wk, f).T)),
        "w8v": _to_fp8(_tile_kp(np.asarray(wv, f).T)),
        "w8o": _to_fp8(_tile_kp(np.asarray(wo, f).T)),
        "bqv": _col_pc(np.asarray(bq, f) * sc, DC),
        "bkv": _col_pc(bk, DC),
        "bov": _col_pc(bo, DC),
        "bvv": c(np.asarray(bv, f).reshape(1, D)),
        "f1b": _col_pc(fc1_b, FFC),
        "f2b": _col_pc(fc2_b, DC),
        "lnrows": _to_bf16(lnrows),
        "ones_col": np.ones((P, 1), f),
        "ones_row": np.ones((1, T), f),
    }
    f1T = np.asarray(fc1_w, f).T           # [D, FF]
    f2T = np.asarray(fc2_w, f).T           # [FF, D]
    if mlp_fp8:
        # fc1: [FFC, P, KP, 2, P]; fc2: [FFC//2, P, 2, D] (ff = fo*P + p)
        shared["fc1_t"] = _to_fp8(
            f1T.reshape(KP, 2, P, FFC, P).transpose(3, 2, 0, 1, 4))
        shared["fc2_t"] = _to_fp8(
            f2T.reshape(FFC // 2, 2, P, D).transpose(0, 2, 1, 3))
    else:
        shared["fc1_t"] = _to_bf16(
            f1T.reshape(DC, P, FFC, P).transpose(2, 1, 0, 3))
        shared["fc2_t"] = _to_bf16(f2T.reshape(FFC, P, D))
    return shared


def _prep_batch(query_b, key_b):
    """Per-batch tensors: query_b [T, D], key_b [S, D]."""
    f = np.float32
    qT = np.asarray(query_b, f).T          # [D, T]
    kT = np.asarray(key_b, f).T            # [D, S]
    return (
        _to_fp8(qT.reshape(KP, 2, P, T).transpose(2, 0, 1, 3)),
        np.ascontiguousarray(qT.reshape(DC, P, T).transpose(1, 0, 2)),
        _to_fp8(kT.reshape(KP, 2, P, S).transpose(2, 0, 1, 3)),
    )


def kernel(query, key, wq, bq, wk, bk, wv, bv, wo, bo,
           ln2_g, ln2_b, fc1_w, fc1_b, fc2_w, fc2_b, lnp_g, lnp_b):
    from concourse.bass_utils import run_bass_kernel_spmd

    query = np.asarray(query, np.float32)
    key = np.asarray(key, np.float32)
    use_bv = bool(np.any(np.asarray(bv)))
    nc = _get_nc(use_bv, MLP_FP8)

    shared = _prep_shared(wq, bq, wk, bk, wv, bv, wo, bo,
                          ln2_g, ln2_b, fc1_w, fc1_b, fc2_w, fc2_b,
                          lnp_g, lnp_b, MLP_FP8)
    in_maps = []
    for core in range(NCORES):
        m = dict(shared)
        q8s, qfs, k8s = [], [], []
        for j in range(BPC):
            b = core * BPC + j
            q8, qf, k8 = _prep_batch(query[b], key[b])
            q8s.append(q8)
            qfs.append(qf)
            k8s.append(k8)
        m["qT8"] = np.stack(q8s)
        m["qTf"] = np.stack(qfs)
        m["kT8"] = np.stack(k8s)
        in_maps.append(m)

    res = run_bass_kernel_spmd(nc, in_maps, core_ids=list(range(NCORES)))
    kernel._last_result = res
    out = np.stack([r["out"] for r in res.results])   # [NC, BPC, P, DC, T]
    # [core, b, p, c, t] -> [B, T, c*P+p]
    out = out.reshape(B, P, DC, T).transpose(0, 3, 2, 1).reshape(B, T, D)
    return np.ascontiguousarray(out)
